# revision 1
# baseline (speedup 1.0000x reference)
"""Trainium2 Bass kernel: BoundaryActivation.

Per sample: x1 = cummax(x, H), x2 = reverse-cummax(x, H), x3 = cummax(x, W),
x4 = reverse-cummax(x, W); out = conv1x1(concat([x, x1, x2, x3, x4])) + bias.

Sharding: data-parallel over batch, B=8 -> 8 NeuronCores, one sample each.

Per-core algorithm (sample x_s [256, 128, 128], flattened to [256, 16384]):
  - channel-in-partition layout [c_chunk(128), (h, w)]; matmul contracts
    channels (fp32r, full PE rate), N tiles of 512 spatial positions.
  - W-direction scans (x3/x4): tensor_tensor_scan along the free axis with a
    -inf "reset" bias every 128 elements (row starts); reverse via negative
    stride APs.
  - H-direction scans (x1/x2): strided-AP copy gathers a transposed band
    xT [c, (w8, h64)], scan along free with resets every 64, matmul in
    transposed spatial order into a separate PSUM group; the PSUM->SBUF copy
    un-transposes via a strided destination AP (free).
  - H is processed in two 64-row phases so SBUF holds only half maps. Suffix
    carries for the top half come from a column-max pre-pass over the bottom
    half; prefix carries for the bottom half come from the top half's last
    scan row. Carries are applied as an elementwise max AFTER the local scan
    (prefix-max with seed == max(unseeded prefix-max, seed)).
"""

import numpy as np
from contextlib import ExitStack

import concourse.bass as bass
import concourse.bacc as bacc
import concourse.mybir as mybir
import concourse.tile as tile
from concourse.bass_utils import run_bass_kernel_spmd

F32 = mybir.dt.float32
F32R = mybir.dt.float32r
AL = mybir.AluOpType
AFT = mybir.ActivationFunctionType

NEG = -3.0e38  # effective -inf for scan resets / initials

B = 8
C = 256
H = 128
W = 128
O = 256
NCC = 2          # channel chunks of 128
NQ = 2           # output-channel chunks of 128
HALF = 64        # rows per phase
BAND = 512       # matmul N-tile (spatial positions)
NBAND = 16       # hw bands per phase  (4 rows x 128 w each)
NTB = 16         # T bands per phase   (8 cols x 64 h each)
PRE_CHUNK = 1024  # pre-pass rows chunk (8 rows x 128 w)

# map index: 0=x, 1=x1(cummax H), 2=x2(revcummax H), 3=x3(cummax W), 4=x4(revcummax W)


def _w_col(m, cc, q):
    return ((m * NCC + cc) * NQ + q) * 128


def build_program():
    nc = bacc.Bacc()
    x_d = nc.declare_dram_parameter("x", [C, H * W], F32, isOutput=False)
    w_d = nc.declare_dram_parameter("wT", [5 * C, O], F32, isOutput=False)
    b_d = nc.declare_dram_parameter("bias", [O, 1], F32, isOutput=False)
    mA_d = nc.declare_dram_parameter("maskA", [128, BAND], F32, isOutput=False)
    mB_d = nc.declare_dram_parameter("maskB", [128, BAND], F32, isOutput=False)
    out_d = nc.declare_dram_parameter("out", [O, H * W], F32, isOutput=True)

    with ExitStack() as ctx:
        tc = ctx.enter_context(tile.TileContext(nc))

        const = ctx.enter_context(tc.tile_pool(name="const", bufs=1))
        persist = ctx.enter_context(tc.tile_pool(name="persist", bufs=1))
        xhalf_p = ctx.enter_context(tc.tile_pool(name="xhalf", bufs=2))
        tsurf_p = ctx.enter_context(tc.tile_pool(name="tsurf", bufs=1))
        stream_p = ctx.enter_context(tc.tile_pool(name="stream", bufs=2))
        pp_p = ctx.enter_context(tc.tile_pool(name="pp", bufs=4))
        xT_p = ctx.enter_context(tc.tile_pool(name="xT", bufs=2))
        x1T_p = ctx.enter_context(tc.tile_pool(name="x1T", bufs=3))
        x2T_p = ctx.enter_context(tc.tile_pool(name="x2T", bufs=3))
        x3_p = ctx.enter_context(tc.tile_pool(name="x3", bufs=3))
        x4_p = ctx.enter_context(tc.tile_pool(name="x4", bufs=3))
        outsb_p = ctx.enter_context(tc.tile_pool(name="outsb", bufs=4))
        psum_hw = ctx.enter_context(tc.tile_pool(name="psum_hw", bufs=4, space="PSUM"))
        psum_t = ctx.enter_context(tc.tile_pool(name="psum_t", bufs=4, space="PSUM"))

        # ---- constants ----
        wstage = const.tile([128, 20 * 128], F32, tag="wstage")
        # one DMA: wT[(tk p) o] -> [p, tk, o]; w_sb col layout tk*256 + q*128
        nc.sync.dma_start(
            wstage[:].rearrange("p (tk o) -> p tk o", o=O),
            w_d[:].rearrange("(tk p) o -> p tk o", p=128))
        w_sb = const.tile([128, 20 * 128], F32R, tag="w_sb")
        nc.scalar.activation(w_sb[:], wstage[:], AFT.Copy)
        maskA = const.tile([128, BAND], F32, tag="maskA")
        nc.sync.dma_start(maskA[:], mA_d[:])
        maskB = const.tile([128, BAND], F32, tag="maskB")
        nc.sync.dma_start(maskB[:], mB_d[:])
        bias_sb = const.tile([128, NQ], F32, tag="bias_sb")
        for q in range(NQ):
            nc.sync.dma_start(bias_sb[:, q:q + 1], b_d[q * 128:(q + 1) * 128, :])

        # carry tiles: column maxes per (chunk)  [128c, 128w]
        cmA = [persist.tile([128, W], F32R, tag=f"cmA{cc}", name=f"cmA{cc}") for cc in range(NCC)]
        cmB = [persist.tile([128, W], F32R, tag=f"cmB{cc}", name=f"cmB{cc}") for cc in range(NCC)]

        def w_ap(m, cc, q):
            return w_sb[:, _w_col(m, cc, q):_w_col(m, cc, q) + 128]

        # ---- pre-pass: column max of bottom half -> cmB ----
        for cc in range(NCC):
            nchunks = HALF * W // PRE_CHUNK  # 8
            acc = None
            for j in range(nchunks):
                t = stream_p.tile([128, PRE_CHUNK], F32, tag="stream", name="stream")
                src = x_d[cc * 128:(cc + 1) * 128,
                          HALF * W + j * PRE_CHUNK:HALF * W + (j + 1) * PRE_CHUNK]
                nc.gpsimd.dma_start(t[:], src)
                part = pp_p.tile([128, W], F32R, tag="pp", name="pp")
                # view (w outer, h inner); X-reduce over h
                v = t[:].rearrange("p (h w) -> p w h", w=W)
                nc.vector.tensor_reduce(part[:], v, mybir.AxisListType.X, AL.max)
                if acc is None:
                    acc = part
                else:
                    nc.vector.tensor_max(part[:], part[:], acc[:])
                    acc = part
            nc.vector.tensor_copy(cmB[cc][:], acc[:])

        # ---- phases ----
        for phase in ("A", "B"):
            h_off = 0 if phase == "A" else HALF
            col0 = h_off * W  # dram column offset of this phase

            xh = []
            for cc in range(NCC):
                t = xhalf_p.tile([128, HALF * W], F32, tag="xh", name="xh")
                nc.gpsimd.dma_start(
                    t[:], x_d[cc * 128:(cc + 1) * 128, col0:col0 + HALF * W])
                xh.append(t)

            tsurf = [tsurf_p.tile([128, HALF * W], F32, tag=f"ts{q}", name=f"ts{q}")
                     for q in range(NQ)]

            # ---- T path: x1 (cummax H), x2 (reverse cummax H) ----
            for tb in range(NTB):
                w0 = tb * 8
                x1T = {}
                x2T = {}
                xTd = {}
                for cc in range(NCC):
                    xT = xT_p.tile([128, BAND], F32R, tag="xT")
                    # gather transposed band: free = (w 8, h 64)
                    src = xh[cc][:].rearrange("p (h w) -> p w h", w=W)[:, w0:w0 + 8, :]
                    nc.scalar.activation(
                        xT[:].rearrange("p (w h) -> p w h", h=HALF), src, AFT.Copy)

                    t1 = x1T_p.tile([128, BAND], F32R, tag="x1T")
                    nc.vector.tensor_tensor_scan(
                        t1[:], maskB[:], xT[:], NEG, AL.add, AL.max)
                    t2 = x2T_p.tile([128, BAND], F32R, tag="x2T")
                    nc.vector.tensor_tensor_scan(
                        t2[:, ::-1], maskB[:], xT[:, ::-1], NEG, AL.add, AL.max)

                    if phase == "A":
                        # seed suffix-max with bottom-half column max
                        nc.vector.tensor_max(
                            t2[:].rearrange("p (w h) -> p w h", h=HALF),
                            t2[:].rearrange("p (w h) -> p w h", h=HALF),
                            cmB[cc][:, w0:w0 + 8].broadcast_to((128, 8, HALF)))
                        # harvest top-half column max for phase B prefix seed
                        nc.scalar.activation(
                            cmA[cc][:, w0:w0 + 8],
                            t1[:, HALF - 1::HALF], AFT.Copy)
                    else:
                        # seed prefix-max with top-half column max
                        nc.vector.tensor_max(
                            t1[:].rearrange("p (w h) -> p w h", h=HALF),
                            t1[:].rearrange("p (w h) -> p w h", h=HALF),
                            cmA[cc][:, w0:w0 + 8].broadcast_to((128, 8, HALF)))
                    x1T[cc] = t1
                    x2T[cc] = t2
                    xTd[cc] = xT

                for q in range(NQ):
                    pt = psum_t.tile([128, BAND], F32, tag="pt")
                    terms = [(0, 0, xTd[0]), (0, 1, xTd[1]),
                             (1, 0, x1T[0]), (1, 1, x1T[1]),
                             (2, 0, x2T[0]), (2, 1, x2T[1])]
                    for i, (m, cc, rhs) in enumerate(terms):
                        nc.tensor.matmul(
                            pt[:], w_ap(m, cc, q), rhs[:],
                            start=(i == 0), stop=(i == len(terms) - 1))
                    # un-transpose while copying PSUM -> SBUF surface
                    dst = tsurf[q][:].rearrange("p (h w) -> p w h", w=W)[:, w0:w0 + 8, :]
                    nc.scalar.activation(
                        dst, pt[:].rearrange("p (w h) -> p w h", h=HALF), AFT.Copy)

            # ---- hw path: x, x3 (cummax W), x4 (reverse cummax W) ----
            for b in range(NBAND):
                c0 = b * BAND
                x3 = {}
                x4 = {}
                for cc in range(NCC):
                    t3 = x3_p.tile([128, BAND], F32R, tag="x3")
                    nc.vector.tensor_tensor_scan(
                        t3[:], maskA[:], xh[cc][:, c0:c0 + BAND],
                        NEG, AL.add, AL.max)
                    t4 = x4_p.tile([128, BAND], F32R, tag="x4")
                    nc.vector.tensor_tensor_scan(
                        t4[:, ::-1], maskA[:], xh[cc][:, c0:c0 + BAND][:, ::-1],
                        NEG, AL.add, AL.max)
                    x3[cc] = t3
                    x4[cc] = t4

                for q in range(NQ):
                    ph = psum_hw.tile([128, BAND], F32, tag="ph")
                    terms = [(3, 0, x3[0][:]), (3, 1, x3[1][:]),
                             (4, 0, x4[0][:]), (4, 1, x4[1][:])]
                    for i, (m, cc, rhs) in enumerate(terms):
                        nc.tensor.matmul(
                            ph[:], w_ap(m, cc, q), rhs,
                            start=(i == 0), stop=(i == len(terms) - 1))
                    osb = outsb_p.tile([128, BAND], F32, tag="osb")
                    # out = (psum_hw + bias) + tsurf
                    nc.vector.scalar_tensor_tensor(
                        osb[:], ph[:], bias_sb[:, q:q + 1],
                        tsurf[q][:, c0:c0 + BAND], AL.add, AL.add)
                    nc.gpsimd.dma_start(
                        out_d[q * 128:(q + 1) * 128, col0 + c0:col0 + c0 + BAND],
                        osb[:])

    nc.finalize()
    return nc


_PROGRAM = None


def _get_program():
    global _PROGRAM
    if _PROGRAM is None:
        _PROGRAM = build_program()
    return _PROGRAM


def make_masks():
    mA = np.zeros((128, BAND), dtype=np.float32)
    mA[:, 0::128] = NEG
    mB = np.zeros((128, BAND), dtype=np.float32)
    mB[:, 0::64] = NEG
    return mA, mB


def make_in_maps(x, conv_w, conv_b):
    wT = np.ascontiguousarray(conv_w.T).astype(np.float32)      # [1280, 256]
    bias = conv_b.reshape(O, 1).astype(np.float32)
    mA, mB = make_masks()
    in_maps = []
    for i in range(B):
        in_maps.append({
            "x": np.ascontiguousarray(x[i].reshape(C, H * W)).astype(np.float32),
            "wT": wT,
            "bias": bias,
            "maskA": mA,
            "maskB": mB,
        })
    return in_maps


def kernel(x, conv_w, conv_b):
    nc = _get_program()
    in_maps = make_in_maps(x, conv_w, conv_b)
    res = run_bass_kernel_spmd(nc, in_maps, core_ids=list(range(B)))
    outs = [res.results[i]["out"].reshape(O, H, W) for i in range(B)]
    return np.stack(outs, axis=0).astype(np.float32)

